# revision 1
# baseline (speedup 1.0000x reference)
"""GatedGraphNN Trainium2 kernel: 8-core SPMD, node-sharded with per-step AllGather.

Algorithm notes:
  - messages = h[col] @ W_msg.T + b_msg ; agg = segsum(messages, row). Linearity:
    agg @ W_ih.T = raw @ (W_ih W_msg).T + outer(deg, W_ih b_msg), raw = segsum(h[col]).
    So no per-edge matmul: gather h[col] (bf16), segment-sum via one-hot matmuls on PE,
    then fused dense GRU with W_c = W_ih @ W_msg.
  - Each core owns 12800 dest nodes (N padded 100000->102400). Edges bucketed by
    128-dest "quarters"; each quarter's edges are padded to a multiple of 128 slots with
    the chunk count shared across cores (SPMD-uniform program).
  - h kept bf16: node-major replica h_full [102400,128] in DRAM (AllGather output each
    step) for gathers; feature-major shard resident in SBUF for the W_hh matmul.
  - Epilogue (gates, blend) computed with fp32 intermediates; h state rounds to bf16
    once per step. Final step emits fp32.
"""

import numpy as np
import ml_dtypes

BF16 = ml_dtypes.bfloat16
N, H, STEPS, NCORES = 100000, 128, 5, 8
NPAD = 102400
SHARD = NPAD // NCORES          # 12800
NT = SHARD // 512               # 25 dense tiles of 512 dests
NQ = SHARD // 128               # 100 quarters of 128 dests
CHUNK = 128


def _preprocess(edge_index):
    """Host-side tables. Slots sorted by (dense-tile T, src residue r=col%4, dest-qtr dq);
    each (T,r,dq) cell padded to a shared (cross-core max) chunk count. Gather uses
    dma_gather with stride-4 source rows (idx = col//4 < 25600 fits int16)."""
    row = np.asarray(edge_index[0]).astype(np.int64)
    col = np.asarray(edge_index[1]).astype(np.int64)
    core = row // SHARD
    rloc = row - core * SHARD
    T = rloc // 512
    dq = (rloc % 512) // 128
    off = rloc % 128
    res = col % 4
    cell = (T * 4 + res) * 4 + dq          # (T, res, dq) -> 0..399  ... NT*16 cells
    NCELL = NT * 16
    cnt = np.zeros((NCORES, NCELL), np.int64)
    np.add.at(cnt, (core, cell), 1)
    Ccell = np.maximum(1, (cnt.max(axis=0) + CHUNK - 1) // CHUNK)
    cstart = np.zeros(NCELL + 1, np.int64)
    cstart[1:] = np.cumsum(Ccell)
    TOT = int(cstart[-1])

    idx16 = np.zeros((NCORES, 128, TOT * 8), np.int16)   # [16-wrap, replicated x8]
    offs = np.full((NCORES, 128, TOT), 999.0, np.float32)
    deg = np.zeros((NCORES, SHARD), np.float32)

    order = np.lexsort((col, cell, core))
    core_s, cell_s, off_s, col_s, rloc_s = (
        core[order], cell[order], off[order], col[order], rloc[order])
    np.add.at(deg, (core_s, rloc_s), 1.0)

    key = core_s * NCELL + cell_s
    bounds = np.flatnonzero(np.diff(key)) + 1
    starts = np.concatenate([[0], bounds])
    ends = np.concatenate([bounds, [len(key)]])
    idxflat = np.zeros((NCORES, TOT * 128), np.int16)
    for st, en in zip(starts, ends):
        c = int(core_s[st]); ce = int(cell_s[st]); n = en - st
        s0 = int(cstart[ce]) * 128
        sl = np.arange(n)
        idxflat[c, s0 + sl] = (col_s[st:en] // 4).astype(np.int16)
        offs[c, (sl % 128), int(cstart[ce]) + sl // 128] = off_s[st:en]
    # wrap into 16 partitions: slot i -> [i%16, i//16]; replicate to 128 partitions
    for c in range(NCORES):
        w = idxflat[c].reshape(TOT * 8, 16).T          # [16, TOT*8]
        idx16[c] = np.tile(w, (8, 1))
    pad_frac = (TOT * 128 * NCORES - len(row)) / len(row)
    return dict(idx16=idx16, offs=offs.astype(BF16), deg=deg, Ccell=Ccell,
                cstart=cstart, TOT=TOT, pad_frac=pad_frac)


def _build(Ccell, cstart, TOT):
    import concourse.bass as bass
    import concourse.bacc as bacc
    import concourse.mybir as mybir
    import concourse.tile as tile
    from concourse.bass import IndirectOffsetOnAxis, broadcast_tensor_aps

    dt = mybir.dt
    AF = mybir.ActivationFunctionType
    OP = mybir.AluOpType
    nc = bacc.Bacc(num_devices=NCORES)
    RG = [list(range(NCORES))]

    x_nm = nc.dram_tensor("x_nm", [SHARD, H], dt.bfloat16, kind="ExternalInput")
    x_T = nc.dram_tensor("x_T", [H, SHARD], dt.bfloat16, kind="ExternalInput")
    idx_d = nc.dram_tensor("idx", [128, TOT * 8], dt.int16, kind="ExternalInput")
    offs_d = nc.dram_tensor("offs", [128, TOT], dt.bfloat16, kind="ExternalInput")
    deg_d = nc.dram_tensor("deg", [1, SHARD], dt.bfloat16, kind="ExternalInput")
    wct_d = nc.dram_tensor("wct", [H, 3 * H], dt.bfloat16, kind="ExternalInput")
    whht_d = nc.dram_tensor("whht", [H, 3 * H], dt.bfloat16, kind="ExternalInput")
    v3_d = nc.dram_tensor("v3", [1, 3 * H], dt.bfloat16, kind="ExternalInput")
    bias_d = nc.dram_tensor("bias", [H, 4], dt.float32, kind="ExternalInput")
    iota_d = nc.dram_tensor("iota", [H, H], dt.bfloat16, kind="ExternalInput")
    idn_d = nc.dram_tensor("idn", [H, H], dt.bfloat16, kind="ExternalInput")
    idnf_d = nc.dram_tensor("idnf", [H, H], dt.float32, kind="ExternalInput")
    out_d = nc.dram_tensor("out", [SHARD, H], dt.float32, kind="ExternalOutput")

    h_full = nc.dram_tensor("h_full", [NPAD, H], dt.bfloat16, kind="Internal",
                            addr_space="Shared")
    bounce = nc.dram_tensor("bounce", [SHARD, H], dt.bfloat16, kind="Internal")

    with tile.TileContext(nc) as tc:
        with (
            tc.tile_pool(name="res", bufs=1) as res,
            tc.tile_pool(name="gath", bufs=2) as gpool,
            tc.tile_pool(name="oh", bufs=2) as ohpool,
            tc.tile_pool(name="agg", bufs=2) as apool,
            tc.tile_pool(name="epi", bufs=2) as epool,
            tc.tile_pool(name="stg", bufs=2) as spool,
            tc.tile_pool(name="pseg", bufs=2, space="PSUM") as pseg,
            tc.tile_pool(name="pden", bufs=1, space="PSUM") as pden,
            tc.tile_pool(name="ptr", bufs=1, space="PSUM") as ptr,
        ):
            def ld(dram, shape, dtype, name):
                t = res.tile(shape, dtype, tag=name)
                nc.sync.dma_start(t[:], dram[:, :])
                return t

            idx_sb = ld(idx_d, [128, TOT * 8], dt.int16, "idx")
            offs_sb = ld(offs_d, [128, TOT], dt.bfloat16, "offs")
            deg_sb = ld(deg_d, [1, SHARD], dt.bfloat16, "deg")
            wct = ld(wct_d, [H, 3 * H], dt.bfloat16, "wct")
            whht = ld(whht_d, [H, 3 * H], dt.bfloat16, "whht")
            v3 = ld(v3_d, [1, 3 * H], dt.bfloat16, "v3")
            bias = ld(bias_d, [H, 4], dt.float32, "bias")
            iota = ld(iota_d, [H, H], dt.bfloat16, "iota")
            idn = ld(idn_d, [H, H], dt.bfloat16, "idn")
            idnf = ld(idnf_d, [H, H], dt.float32, "idnf")

            hT = [res.tile([H, SHARD], dt.bfloat16, tag=f"hT{i}", name=f"hT{i}")
                  for i in range(2)]
            nc.sync.dma_start(hT[0][:], x_T[:, :])

            # initial replica of h
            nc.sync.dma_start(bounce[:, :], x_nm[:, :])
            nc.gpsimd.collective_compute(
                "AllGather", OP.bypass, replica_groups=RG,
                ins=[bounce[:, :]], outs=[h_full[:, :]])

            b_r = bias[:, 0:1]
            b_z = bias[:, 1:2]
            b_in = bias[:, 2:3]
            b_hn = bias[:, 3:4]

            for s in range(STEPS):
                hcur, hnxt = hT[s % 2], hT[(s + 1) % 2]
                last = s == STEPS - 1
                for T in range(NT):
                    c0 = int(cstart[T * 16])
                    CT = int(cstart[T * 16 + 16] - c0)
                    g = gpool.tile([128, CT, H], dt.bfloat16, tag="gath")
                    for r in range(4):
                        rc0 = int(cstart[T * 16 + r * 4])
                        rk = int(cstart[T * 16 + r * 4 + 4] - rc0)
                        n_idx = rk * 128
                        nc.gpsimd.dma_gather(
                            g[:, rc0 - c0:rc0 - c0 + rk, :],
                            h_full[r::4, :],
                            idx_sb[:, (rc0 * 128) // 16:((rc0 + rk) * 128) // 16],
                            n_idx, n_idx, H, elem_step=4 * H)
                    oh = ohpool.tile([128, CT * 128], dt.bfloat16, tag="oh")
                    a_in, b_in2 = broadcast_tensor_aps(
                        offs_sb[:, c0:c0 + CT, None], iota[:, None, :])
                    nc.vector.tensor_tensor(
                        oh[:].rearrange("p (c f) -> p c f", c=CT), a_in, b_in2,
                        OP.is_equal)
                    ps = pseg.tile([H, 512], dt.float32, tag="pseg")
                    for j in range(4):
                        first = True
                        nmm = sum(int(Ccell[T * 16 + r * 4 + j]) for r in range(4))
                        done = 0
                        for r in range(4):
                            ch0 = int(cstart[T * 16 + r * 4 + j] - c0)
                            for k in range(int(Ccell[T * 16 + r * 4 + j])):
                                cc = ch0 + k
                                done += 1
                                nc.tensor.matmul(
                                    ps[:, j * 128:(j + 1) * 128],
                                    g[:, cc, :],
                                    oh[:, cc * 128:(cc + 1) * 128],
                                    start=first, stop=(done == nmm))
                                first = False
                    ragg = apool.tile([H, 512], dt.bfloat16, tag="ragg")
                    nc.scalar.copy(ragg[:], ps[:])

                    hsl = hcur[:, T * 512:(T + 1) * 512]
                    dsl = deg_sb[0:1, T * 512:(T + 1) * 512]
                    p_r = pden.tile([H, 512], dt.float32, tag="p_r")
                    p_z = pden.tile([H, 512], dt.float32, tag="p_z")
                    p_in = pden.tile([H, 512], dt.float32, tag="p_in")
                    p_hn = pden.tile([H, 512], dt.float32, tag="p_hn")
                    nc.tensor.matmul(p_r[:], wct[:, 0:128], ragg[:], start=True, stop=False)
                    nc.tensor.matmul(p_r[:], whht[:, 0:128], hsl, start=False, stop=False)
                    nc.tensor.matmul(p_r[:], v3[0:1, 0:128], dsl, start=False, stop=True)
                    nc.tensor.matmul(p_z[:], wct[:, 128:256], ragg[:], start=True, stop=False)
                    nc.tensor.matmul(p_z[:], whht[:, 128:256], hsl, start=False, stop=False)
                    nc.tensor.matmul(p_z[:], v3[0:1, 128:256], dsl, start=False, stop=True)
                    nc.tensor.matmul(p_in[:], wct[:, 256:384], ragg[:], start=True, stop=False)
                    nc.tensor.matmul(p_in[:], v3[0:1, 256:384], dsl, start=False, stop=True)
                    nc.tensor.matmul(p_hn[:], whht[:, 256:384], hsl, start=True, stop=True)

                    r = epool.tile([H, 512], dt.float32, tag="r")
                    z = epool.tile([H, 512], dt.float32, tag="z")
                    ghn = epool.tile([H, 512], dt.float32, tag="ghn")
                    t2 = epool.tile([H, 512], dt.float32, tag="t2", bufs=1)
                    pre_n = epool.tile([H, 512], dt.float32, tag="pre_n", bufs=1)
                    nn = epool.tile([H, 512], dt.float32, tag="nn")
                    am = epool.tile([H, 512], dt.float32, tag="am", bufs=1)
                    bm = epool.tile([H, 512], dt.float32, tag="bm", bufs=1)

                    nc.scalar.activation(r[:], p_r[:], AF.Sigmoid, bias=b_r)
                    nc.scalar.activation(z[:], p_z[:], AF.Sigmoid, bias=b_z)
                    nc.scalar.activation(ghn[:], p_hn[:], AF.Identity, bias=b_hn)
                    nc.vector.scalar_tensor_tensor(
                        t2[:], r[:], 0.0, ghn[:], OP.add, OP.mult)
                    nc.vector.tensor_tensor(pre_n[:], t2[:], p_in[:], OP.add)
                    nc.scalar.activation(nn[:], pre_n[:], AF.Tanh, bias=b_in)
                    nc.vector.tensor_tensor(am[:], hsl, nn[:], OP.subtract)
                    nc.vector.scalar_tensor_tensor(
                        bm[:], z[:], 0.0, am[:], OP.add, OP.mult)

                    if not last:
                        hn_sl = hnxt[:, T * 512:(T + 1) * 512]
                        nc.vector.tensor_tensor(hn_sl, bm[:], nn[:], OP.add)
                        stg = spool.tile([128, 4, H], dt.bfloat16, tag="stg")
                        for j in range(4):
                            pt = ptr.tile([128, 128], dt.bfloat16, tag="pt")
                            nc.tensor.transpose(
                                pt[:], hnxt[:, T * 512 + j * 128: T * 512 + (j + 1) * 128],
                                idn[:])
                            nc.scalar.copy(stg[:, j, :], pt[:])
                        nc.sync.dma_start(
                            bounce.rearrange("(t g p) f -> t p g f", p=128, g=4)[T],
                            stg[:])
                    else:
                        hf = epool.tile([H, 512], dt.float32, tag="hf", bufs=2)
                        nc.vector.tensor_tensor(hf[:], bm[:], nn[:], OP.add)
                        stgf = spool.tile([128, 4, H], dt.float32, tag="stgf")
                        for j in range(4):
                            ptf = ptr.tile([128, 128], dt.float32, tag="ptf")
                            nc.tensor.matmul(ptf[:], hf[:, j * 128:(j + 1) * 128],
                                             idnf[:], is_transpose=True)
                            nc.scalar.copy(stgf[:, j, :], ptf[:])
                        nc.sync.dma_start(
                            out_d.rearrange("(t g p) f -> t p g f", p=128, g=4)[T],
                            stgf[:])
                if not last:
                    nc.gpsimd.collective_compute(
                        "AllGather", OP.bypass, replica_groups=RG,
                        ins=[bounce[:, :]], outs=[h_full[:, :]])
    nc.finalize()
    return nc


_CACHE = {}


def kernel(**inputs):
    x = np.asarray(inputs["x"], np.float32)
    edge_index = np.asarray(inputs["edge_index"])
    W_msg = np.asarray(inputs["W_msg"], np.float32)
    b_msg = np.asarray(inputs["b_msg"], np.float32)
    W_ih = np.asarray(inputs["W_ih"], np.float32)
    W_hh = np.asarray(inputs["W_hh"], np.float32)
    b_ih = np.asarray(inputs["b_ih"], np.float32)
    b_hh = np.asarray(inputs["b_hh"], np.float32)

    pp = _preprocess(edge_index)
    key = (pp["TOT"], tuple(pp["Ccell"].tolist()))
    if key not in _CACHE:
        _CACHE[key] = _build(pp["Ccell"], pp["cstart"], pp["TOT"])
    nc = _CACHE[key]

    xp = np.zeros((NPAD, H), np.float32)
    xp[:N] = x
    W_c = W_ih @ W_msg
    v3 = (W_ih @ b_msg).reshape(1, 3 * H)
    bias = np.stack([
        b_ih[0:128] + b_hh[0:128],
        b_ih[128:256] + b_hh[128:256],
        b_ih[256:384],
        b_hh[256:384],
    ], axis=1).astype(np.float32)
    iota = np.broadcast_to(np.arange(H, dtype=np.float32), (H, H)).astype(BF16)
    idn = np.eye(H, dtype=np.float32)

    in_maps = []
    for c in range(NCORES):
        sh = xp[c * SHARD:(c + 1) * SHARD]
        in_maps.append({
            "x_nm": np.ascontiguousarray(sh).astype(BF16),
            "x_T": np.ascontiguousarray(sh.T).astype(BF16),
            "idx": pp["idx16"][c],
            "offs": pp["offs"][c],
            "deg": pp["deg"][c].reshape(1, SHARD).astype(BF16),
            "wct": np.ascontiguousarray(W_c.T).astype(BF16),
            "whht": np.ascontiguousarray(W_hh.T).astype(BF16),
            "v3": v3.astype(BF16),
            "bias": bias,
            "iota": np.ascontiguousarray(iota),
            "idn": idn.astype(BF16),
            "idnf": idn,
        })

    global _last_in_maps
    _last_in_maps = in_maps
    from concourse.bass_utils import run_bass_kernel_spmd
    res = run_bass_kernel_spmd(nc, in_maps, core_ids=list(range(NCORES)))
    outs = res.results
    full = np.concatenate([outs[c]["out"] for c in range(NCORES)], axis=0)
    return full[:N].astype(np.float32)



# revision 2
# speedup vs baseline: 7.0556x; 7.0556x over previous
"""GatedGraphNN Trainium2 kernel v2: 8-core SPMD, node-sharded, per-step AllGather.

Improvements over v1:
  - 4 SWDGE queues: gather descriptor generation (the v1 bottleneck, ~4.5us/call
    + ~4.3ns/idx serial on Q7) is spread across queues (queue = src residue).
  - Optional prepare_only/trigger split: descriptor generation for the next
    tiles overlaps the AllGather wait (Tile defers the h_full RAW dep to the
    trigger).
  - Optional chunked AllGather with double-buffered h_full: the collective is
    split into NCHUNK pipelined AllGathers, each firing as soon as its rows of
    h_next are stored, overlapping the remaining compute of the step.
"""

import numpy as np
import ml_dtypes

BF16 = ml_dtypes.bfloat16
N, H, STEPS, NCORES = 100000, 128, 5, 8
NPAD = 102400
SHARD = NPAD // NCORES          # 12800
NT = SHARD // 512               # 25 dense tiles of 512 dests
CHUNK = 128


def _preprocess(edge_index, nchunk=1):
    """Host-side tables. Nodes laid out in h_full with permutation P:
    chunk-major [chunk, rank, rows-in-chunk, H] so chunked AllGather output is
    contiguous. Slots sorted by (dense-tile T, r=P(col)%4, dest-qtr dq); each
    (T,r,dq) cell padded to a shared (cross-core max) chunk count."""
    row = np.asarray(edge_index[0]).astype(np.int64)
    col = np.asarray(edge_index[1]).astype(np.int64)
    core = row // SHARD
    rloc = row - core * SHARD
    T = rloc // 512
    dq = (rloc % 512) // 128
    off = rloc % 128

    csz = SHARD // nchunk
    score = col // SHARD
    sloc = col - score * SHARD
    schunk = sloc // csz
    soff = sloc - schunk * csz
    p = schunk * (csz * NCORES) + score * csz + soff

    res = p % 4
    cell = (T * 4 + res) * 4 + dq
    NCELL = NT * 16
    cnt = np.zeros((NCORES, NCELL), np.int64)
    np.add.at(cnt, (core, cell), 1)
    Ccell = np.maximum(1, (cnt.max(axis=0) + CHUNK - 1) // CHUNK)
    cstart = np.zeros(NCELL + 1, np.int64)
    cstart[1:] = np.cumsum(Ccell)
    TOT = int(cstart[-1])

    idx16 = np.zeros((NCORES, 128, TOT * 8), np.int16)
    offs = np.full((NCORES, 128, TOT), 999.0, np.float32)
    deg = np.zeros((NCORES, SHARD), np.float32)

    order = np.lexsort((p, cell, core))
    core_s, cell_s, off_s, p_s, rloc_s = (
        core[order], cell[order], off[order], p[order], rloc[order])
    np.add.at(deg, (core_s, rloc_s), 1.0)

    key = core_s * NCELL + cell_s
    bounds = np.flatnonzero(np.diff(key)) + 1
    starts = np.concatenate([[0], bounds])
    ends = np.concatenate([bounds, [len(key)]])
    idxflat = np.zeros((NCORES, TOT * 128), np.int16)
    for st, en in zip(starts, ends):
        c = int(core_s[st]); ce = int(cell_s[st]); n = en - st
        s0 = int(cstart[ce]) * 128
        sl = np.arange(n)
        idxflat[c, s0 + sl] = (p_s[st:en] // 4).astype(np.int16)
        offs[c, (sl % 128), int(cstart[ce]) + sl // 128] = off_s[st:en]
    for c in range(NCORES):
        w = idxflat[c].reshape(TOT * 8, 16).T
        idx16[c] = np.tile(w, (8, 1))
    pad_frac = (TOT * 128 * NCORES - len(row)) / len(row)
    return dict(idx16=idx16, offs=offs.astype(BF16), deg=deg, Ccell=Ccell,
                cstart=cstart, TOT=TOT, pad_frac=pad_frac, nchunk=nchunk)


def _build(Ccell, cstart, TOT, *, nq=4, prepare=True, nchunk=1, steps=STEPS,
           pref=2):
    import concourse.bass as bass
    import concourse.bacc as bacc
    import concourse.mybir as mybir
    import concourse.tile as tile
    from concourse.bass import broadcast_tensor_aps

    dt = mybir.dt
    AF = mybir.ActivationFunctionType
    OP = mybir.AluOpType
    nc = bacc.Bacc(num_devices=NCORES, num_swdge_queues=nq)
    RG = [list(range(NCORES))]

    x_nm = nc.dram_tensor("x_nm", [SHARD, H], dt.bfloat16, kind="ExternalInput")
    x_T = nc.dram_tensor("x_T", [H, SHARD], dt.bfloat16, kind="ExternalInput")
    idx_d = nc.dram_tensor("idx", [128, TOT * 8], dt.int16, kind="ExternalInput")
    offs_d = nc.dram_tensor("offs", [128, TOT], dt.bfloat16, kind="ExternalInput")
    deg_d = nc.dram_tensor("deg", [1, SHARD], dt.bfloat16, kind="ExternalInput")
    wct_d = nc.dram_tensor("wct", [H, 3 * H], dt.bfloat16, kind="ExternalInput")
    whht_d = nc.dram_tensor("whht", [H, 3 * H], dt.bfloat16, kind="ExternalInput")
    v3_d = nc.dram_tensor("v3", [1, 3 * H], dt.bfloat16, kind="ExternalInput")
    bias_d = nc.dram_tensor("bias", [H, 4], dt.float32, kind="ExternalInput")
    iota_d = nc.dram_tensor("iota", [H, H], dt.bfloat16, kind="ExternalInput")
    idn_d = nc.dram_tensor("idn", [H, H], dt.bfloat16, kind="ExternalInput")
    idnf_d = nc.dram_tensor("idnf", [H, H], dt.float32, kind="ExternalInput")
    out_d = nc.dram_tensor("out", [SHARD, H], dt.float32, kind="ExternalOutput")

    nhf = 2 if nchunk > 1 else 1
    h_fulls = [nc.dram_tensor(f"h_full{i}", [NPAD, H], dt.bfloat16,
                              kind="Internal", addr_space="Shared")
               for i in range(nhf)]
    bounce = nc.dram_tensor("bounce", [SHARD, H], dt.bfloat16, kind="Internal")
    CSZ = SHARD // nchunk
    TPC = NT // nchunk
    assert NT % nchunk == 0

    with tile.TileContext(nc) as tc:
        with (
            tc.tile_pool(name="res", bufs=1) as res,
            tc.tile_pool(name="gath", bufs=3) as gpool,
            tc.tile_pool(name="oh", bufs=2) as ohpool,
            tc.tile_pool(name="agg", bufs=2) as apool,
            tc.tile_pool(name="epi", bufs=2) as epool,
            tc.tile_pool(name="stg", bufs=2) as spool,
            tc.tile_pool(name="pseg", bufs=2, space="PSUM") as pseg,
            tc.tile_pool(name="pden", bufs=1, space="PSUM") as pden,
            tc.tile_pool(name="ptr", bufs=1, space="PSUM") as ptr,
        ):
            def ld(dram, shape, dtype, name):
                t = res.tile(shape, dtype, tag=name)
                nc.sync.dma_start(t[:], dram[:, :])
                return t

            idx_sb = ld(idx_d, [128, TOT * 8], dt.int16, "idx")
            offs_sb = ld(offs_d, [128, TOT], dt.bfloat16, "offs")
            deg_sb = ld(deg_d, [1, SHARD], dt.bfloat16, "deg")
            wct = ld(wct_d, [H, 3 * H], dt.bfloat16, "wct")
            whht = ld(whht_d, [H, 3 * H], dt.bfloat16, "whht")
            v3 = ld(v3_d, [1, 3 * H], dt.bfloat16, "v3")
            bias = ld(bias_d, [H, 4], dt.float32, "bias")
            iota = ld(iota_d, [H, H], dt.bfloat16, "iota")
            idn = ld(idn_d, [H, H], dt.bfloat16, "idn")
            idnf = ld(idnf_d, [H, H], dt.float32, "idnf")

            hT = [res.tile([H, SHARD], dt.bfloat16, tag=f"hT{i}", name=f"hT{i}")
                  for i in range(2)]
            nc.sync.dma_start(hT[0][:], x_T[:, :])

            nc.sync.dma_start(bounce[:, :], x_nm[:, :])
            def hfc(i):
                return h_fulls[i].rearrange("(k r) f -> k r f", r=CSZ * NCORES)
            bn_c = bounce.rearrange("(k r) f -> k r f", r=CSZ)
            for k in range(nchunk):
                nc.gpsimd.collective_compute(
                    "AllGather", OP.bypass, replica_groups=RG,
                    ins=[bn_c[k]], outs=[hfc(0)[k]])

            dma_sems = [nc.alloc_semaphore(f"swdge{q}") for q in range(nq)]
            npend = [0] * nq

            b_r = bias[:, 0:1]
            b_z = bias[:, 1:2]
            b_in = bias[:, 2:3]
            b_hn = bias[:, 3:4]

            def emit_preps(s, T):
                hf = h_fulls[s % nhf]
                c0 = int(cstart[T * 16])
                CT = int(cstart[T * 16 + 16] - c0)
                g = gpool.tile([128, CT, H], dt.bfloat16, tag="gath")
                for r in range(4):
                    rc0 = int(cstart[T * 16 + r * 4])
                    rk = int(cstart[T * 16 + r * 4 + 4] - rc0)
                    n_idx = rk * 128
                    q = r % nq
                    kw = dict(queue_num=q) if nq > 1 else {}
                    if prepare:
                        kw.update(prepare_only=True, sem=dma_sems[q])
                        npend[q] += 1
                    nc.gpsimd.dma_gather(
                        g[:, rc0 - c0:rc0 - c0 + rk, :],
                        hf[r::4, :],
                        idx_sb[:, (rc0 * 128) // 16:((rc0 + rk) * 128) // 16],
                        n_idx, n_idx, H, elem_step=4 * H, **kw)
                return g, c0, CT

            def emit_triggers():
                if prepare:
                    for q in range(nq):
                        if npend[q]:
                            nc.gpsimd.trigger_dma(count=None, queue_num=q)
                            npend[q] = 0

            for s in range(steps):
                hcur, hnxt = hT[s % 2], hT[(s + 1) % 2]
                last = s == steps - 1
                pend = {}
                for T in range(min(pref, NT)):
                    pend[T] = emit_preps(s, T)
                for T in range(NT):
                    g, c0, CT = pend.pop(T)
                    emit_triggers()
                    if T + pref < NT:
                        pend[T + pref] = emit_preps(s, T + pref)
                    oh = ohpool.tile([128, CT * 128], dt.bfloat16, tag="oh")
                    a_in, b_in2 = broadcast_tensor_aps(
                        offs_sb[:, c0:c0 + CT, None], iota[:, None, :])
                    nc.vector.tensor_tensor(
                        oh[:].rearrange("p (c f) -> p c f", c=CT), a_in, b_in2,
                        OP.is_equal)
                    ps = pseg.tile([H, 512], dt.float32, tag="pseg")
                    for j in range(4):
                        first = True
                        nmm = sum(int(Ccell[T * 16 + r * 4 + j]) for r in range(4))
                        done = 0
                        for r in range(4):
                            ch0 = int(cstart[T * 16 + r * 4 + j] - c0)
                            for k in range(int(Ccell[T * 16 + r * 4 + j])):
                                cc = ch0 + k
                                done += 1
                                nc.tensor.matmul(
                                    ps[:, j * 128:(j + 1) * 128],
                                    g[:, cc, :],
                                    oh[:, cc * 128:(cc + 1) * 128],
                                    start=first, stop=(done == nmm))
                                first = False
                    ragg = apool.tile([H, 512], dt.bfloat16, tag="ragg")
                    nc.scalar.copy(ragg[:], ps[:])

                    hsl = hcur[:, T * 512:(T + 1) * 512]
                    dsl = deg_sb[0:1, T * 512:(T + 1) * 512]
                    p_r = pden.tile([H, 512], dt.float32, tag="p_r")
                    p_z = pden.tile([H, 512], dt.float32, tag="p_z")
                    p_in = pden.tile([H, 512], dt.float32, tag="p_in")
                    p_hn = pden.tile([H, 512], dt.float32, tag="p_hn")
                    nc.tensor.matmul(p_r[:], wct[:, 0:128], ragg[:], start=True, stop=False)
                    nc.tensor.matmul(p_r[:], whht[:, 0:128], hsl, start=False, stop=False)
                    nc.tensor.matmul(p_r[:], v3[0:1, 0:128], dsl, start=False, stop=True)
                    nc.tensor.matmul(p_z[:], wct[:, 128:256], ragg[:], start=True, stop=False)
                    nc.tensor.matmul(p_z[:], whht[:, 128:256], hsl, start=False, stop=False)
                    nc.tensor.matmul(p_z[:], v3[0:1, 128:256], dsl, start=False, stop=True)
                    nc.tensor.matmul(p_in[:], wct[:, 256:384], ragg[:], start=True, stop=False)
                    nc.tensor.matmul(p_in[:], v3[0:1, 256:384], dsl, start=False, stop=True)
                    nc.tensor.matmul(p_hn[:], whht[:, 256:384], hsl, start=True, stop=True)

                    r = epool.tile([H, 512], dt.float32, tag="r")
                    z = epool.tile([H, 512], dt.float32, tag="z")
                    ghn = epool.tile([H, 512], dt.float32, tag="ghn")
                    t2 = epool.tile([H, 512], dt.float32, tag="t2", bufs=1)
                    pre_n = epool.tile([H, 512], dt.float32, tag="pre_n", bufs=1)
                    nn = epool.tile([H, 512], dt.float32, tag="nn")
                    am = epool.tile([H, 512], dt.float32, tag="am", bufs=1)
                    bm = epool.tile([H, 512], dt.float32, tag="bm", bufs=1)

                    nc.scalar.activation(r[:], p_r[:], AF.Sigmoid, bias=b_r)
                    nc.scalar.activation(z[:], p_z[:], AF.Sigmoid, bias=b_z)
                    nc.scalar.activation(ghn[:], p_hn[:], AF.Identity, bias=b_hn)
                    nc.vector.scalar_tensor_tensor(
                        t2[:], r[:], 0.0, ghn[:], OP.add, OP.mult)
                    nc.vector.tensor_tensor(pre_n[:], t2[:], p_in[:], OP.add)
                    nc.scalar.activation(nn[:], pre_n[:], AF.Tanh, bias=b_in)
                    nc.vector.tensor_tensor(am[:], hsl, nn[:], OP.subtract)
                    nc.vector.scalar_tensor_tensor(
                        bm[:], z[:], 0.0, am[:], OP.add, OP.mult)

                    if not last:
                        hn_sl = hnxt[:, T * 512:(T + 1) * 512]
                        nc.vector.tensor_tensor(hn_sl, bm[:], nn[:], OP.add)
                        stg = spool.tile([128, 4, H], dt.bfloat16, tag="stg")
                        for j in range(4):
                            pt = ptr.tile([128, 128], dt.bfloat16, tag="pt")
                            nc.tensor.transpose(
                                pt[:], hnxt[:, T * 512 + j * 128: T * 512 + (j + 1) * 128],
                                idn[:])
                            nc.scalar.copy(stg[:, j, :], pt[:])
                        nc.sync.dma_start(
                            bounce.rearrange("(t g p) f -> t p g f", p=128, g=4)[T],
                            stg[:])
                        if nchunk > 1 and (T + 1) % TPC == 0:
                            k = (T + 1) // TPC - 1
                            nc.gpsimd.collective_compute(
                                "AllGather", OP.bypass, replica_groups=RG,
                                ins=[bn_c[k]], outs=[hfc((s + 1) % nhf)[k]])
                    else:
                        hf32 = epool.tile([H, 512], dt.float32, tag="hf", bufs=2)
                        nc.vector.tensor_tensor(hf32[:], bm[:], nn[:], OP.add)
                        stgf = spool.tile([128, 4, H], dt.float32, tag="stgf")
                        for j in range(4):
                            ptf = ptr.tile([128, 128], dt.float32, tag="ptf")
                            nc.tensor.matmul(ptf[:], hf32[:, j * 128:(j + 1) * 128],
                                             idnf[:], is_transpose=True)
                            nc.scalar.copy(stgf[:, j, :], ptf[:])
                        nc.sync.dma_start(
                            out_d.rearrange("(t g p) f -> t p g f", p=128, g=4)[T],
                            stgf[:])
                if not last and nchunk == 1:
                    nc.gpsimd.collective_compute(
                        "AllGather", OP.bypass, replica_groups=RG,
                        ins=[bounce[:, :]], outs=[h_fulls[0][:, :]])
    nc.finalize()
    return nc


_CACHE = {}


def make_in_maps(inputs, pp):
    x = np.asarray(inputs["x"], np.float32)
    W_msg = np.asarray(inputs["W_msg"], np.float32)
    b_msg = np.asarray(inputs["b_msg"], np.float32)
    W_ih = np.asarray(inputs["W_ih"], np.float32)
    W_hh = np.asarray(inputs["W_hh"], np.float32)
    b_ih = np.asarray(inputs["b_ih"], np.float32)
    b_hh = np.asarray(inputs["b_hh"], np.float32)

    xp = np.zeros((NPAD, H), np.float32)
    xp[:N] = x
    W_c = W_ih @ W_msg
    v3 = (W_ih @ b_msg).reshape(1, 3 * H)
    bias = np.stack([
        b_ih[0:128] + b_hh[0:128],
        b_ih[128:256] + b_hh[128:256],
        b_ih[256:384],
        b_hh[256:384],
    ], axis=1).astype(np.float32)
    iota = np.broadcast_to(np.arange(H, dtype=np.float32), (H, H)).astype(BF16)
    idn = np.eye(H, dtype=np.float32)

    in_maps = []
    for c in range(NCORES):
        sh = xp[c * SHARD:(c + 1) * SHARD]
        in_maps.append({
            "x_nm": np.ascontiguousarray(sh).astype(BF16),
            "x_T": np.ascontiguousarray(sh.T).astype(BF16),
            "idx": pp["idx16"][c],
            "offs": pp["offs"][c],
            "deg": pp["deg"][c].reshape(1, SHARD).astype(BF16),
            "wct": np.ascontiguousarray(W_c.T).astype(BF16),
            "whht": np.ascontiguousarray(W_hh.T).astype(BF16),
            "v3": v3.astype(BF16),
            "bias": bias,
            "iota": np.ascontiguousarray(iota),
            "idn": idn.astype(BF16),
            "idnf": idn,
        })
    return in_maps


def kernel(**inputs):
    NQ, PREP, NCHUNK = 4, False, 5
    edge_index = np.asarray(inputs["edge_index"])
    pp = _preprocess(edge_index, nchunk=NCHUNK)
    key = (pp["TOT"], tuple(pp["Ccell"].tolist()), NQ, PREP, NCHUNK)
    if key not in _CACHE:
        _CACHE[key] = _build(pp["Ccell"], pp["cstart"], pp["TOT"],
                             nq=NQ, prepare=PREP, nchunk=NCHUNK)
    nc = _CACHE[key]
    in_maps = make_in_maps(inputs, pp)
    global _last_in_maps
    _last_in_maps = in_maps
    from concourse.bass_utils import run_bass_kernel_spmd
    res = run_bass_kernel_spmd(nc, in_maps, core_ids=list(range(NCORES)))
    outs = res.results
    full = np.concatenate([outs[c]["out"] for c in range(NCORES)], axis=0)
    return full[:N].astype(np.float32)
